# revision 17
# baseline (speedup 1.0000x reference)
"""Trainium2 Bass kernel for nn_ImagePatch: scatter-add 64 gathered 16x16x3
patches into each of 32 images [3,512,512], data-parallel over batch on 8
NeuronCores (4 images per core).

Row-interleaved layout: partition p = c*32 + (r mod 32); free dim per
partition = (q = r div 32, col) = 16 blocks x 512 cols. M=32 spans 96
partitions: SDMA engine k serves a fixed partition set (even engines cover
partitions 0-63, odd 64-127), so 96 partitions engage 12 of 16 DMA engines
vs only 8 for a 48-partition M=16 layout — measured ~20% faster end-to-end
on HW despite the G pair table costing 2x the bytes of M=16. A patch at
(r0, c0) touches 16 of the 32 residues per channel, one 16-px run each, in
row-block q0 = r0>>5 (or q0+1 for rows that wrap mod 32). Columns never
cross a boundary (c0+16 <= 512); rows need no scratch block: q0 == 15
implies r0 >= 480 hence p0 <= 16 (no wrapped rows), so the host shifts that
window down one block (reads blocks 14,15) and stores the data in the wrap
slot — the main slot adds zeros to block 14.

- The host assembles G [B, 96, 2N*16]: for patch n, row j -> partition
  (c, (p0+j)%32), slot (2n + wrapped) holds emb[label, c, j, :]; zeros
  in the other slot of the pair (pre-placed AND pre-masked).
- Per patch: ONE tensor_tensor add of a [96, 2, 16] window of T
  (row-blocks q0, q0+1 at col c0) from the static G slot pair at free
  offset 2n*16. The window offset (q0*512 + c0) comes from a register;
  offsets are group-loaded 8 per TensorLoad into two register banks so the
  sequencer issues ~2 instructions per patch instead of 3+.
- Adds split across engines by image (even: Pool/gpsimd, odd: DVE) so two
  images' add chains run concurrently; G loads issue from the SP/sync
  queue so neither adds engine's sequencer gates a later image's G
  arrival (on HW the adds, at ~700ns per register-offset op, are the
  critical path — G starvation of the tail image cost ~25%). T is
  triple-buffered (64KB columns per buf); G streams as half-tiles.

The kernel is DMA (image load/store, G table, offsets) + 64 adds per image.
The program is input-independent (compiled once, any labels/top_left values
work; adds are exact fp32).
"""

import numpy as np

import concourse.bacc as bacc
import concourse.bass as bass
import concourse.mybir as mybir
import concourse.tile as tile
from concourse.bass_utils import run_bass_kernel_spmd

B, N, C, H, W = 32, 64, 3, 512, 512
P = 16
NUM_CLASSES = 128
NCORES = 8
BPC = B // NCORES

M = 32               # row residue -> partition
RB = H // M          # 32 row-blocks in free dim
BANDP = C * M        # 48 partitions
PROW = RB * W        # 16384 elems per partition (no scratch block)
GW = 2 * N * P       # 2048 G elems per partition
GHW = GW // 2        # half-G tile width (patches 0-31 / 32-63)
RLEAD = 6            # register prefetch distance


def build_nc(repeat=1, ablate=(), split=True):
    nc = bacc.Bacc("TRN2", target_bir_lowering=False, enable_asserts=False)
    img_d = nc.dram_tensor("image", [BPC, C, H, W], mybir.dt.float32, kind="ExternalInput")
    g_d = nc.dram_tensor("gtab", [BPC, BANDP, GW], mybir.dt.float32, kind="ExternalInput")
    off_d = nc.dram_tensor("off", [BPC, 1, N], mybir.dt.int32, kind="ExternalInput")
    out_d = nc.dram_tensor("out", [BPC, C, H, W], mybir.dt.float32, kind="ExternalOutput")

    AT = mybir.AluOpType
    with tile.TileContext(nc) as tc:
        with (
            tc.tile_pool(name="big", bufs=4) as bigp,
            tc.tile_pool(name="gp", bufs=3) as gpp,
            tc.tile_pool(name="small", bufs=1) as smallp,
        ):
            vregs = [nc.vector.alloc_register(f"vreg{i}") for i in range(16)]
            pregs = [nc.gpsimd.alloc_register(f"preg{i}") for i in range(16)] if split else None

            import contextlib
            rep_ctx = tc.For_i(0, repeat, 1) if repeat > 1 else contextlib.nullcontext()
            with rep_ctx:
                offall = smallp.tile([1, BPC * N], mybir.dt.int32, tag="off")
                for b in range(BPC):
                    T = bigp.tile([BANDP, PROW], mybir.dt.float32, tag="T")
                    if "dma" not in ablate:
                        Tv = T[:].rearrange("p (rb w) -> p rb w", w=W)
                        for c in range(C):
                            nc.sync.dma_start(
                                out=Tv[c * M:(c + 1) * M, :, :],
                                in_=img_d[b, c].rearrange("(rb pr) w -> pr rb w", pr=M),
                            )
                    if b == 0:
                        # issued after the first image-load DMAs so the big
                        # transfer heads the HWDGE FIFO at kernel start
                        nc.sync.dma_start(out=offall[:],
                                          in_=off_d[:].rearrange("b one n -> one (b n)"))
                    Gh = []
                    for hf in range(2):
                        Gt = gpp.tile([BANDP, GHW], mybir.dt.float32, tag="G")
                        nc.sync.dma_start(out=Gt[:], in_=g_d[b, :, hf * GHW:(hf + 1) * GHW])
                        Gh.append(Gt)
                    offt = offall[:, b * N:(b + 1) * N]

                    if "adds" not in ablate:
                        if split and b % 2 == 0:
                            eng, regs = nc.gpsimd, pregs
                        else:
                            eng, regs = nc.vector, vregs
                        Th = T[:].tensor
                        # group-load 8 offsets per TensorLoad, double-buffered
                        # across two register banks of 8
                        eng.reg_load(regs[0:8], offt[0:1, 0:8])
                        for n in range(N):
                            gidx, i = n // 8, n % 8
                            if i == 0 and n + 8 < N:
                                bank = ((gidx + 1) % 2) * 8
                                eng.reg_load(regs[bank:bank + 8],
                                             offt[0:1, n + 8:n + 16])
                            g = Gh[0] if n < N // 2 else Gh[1]
                            gin = bass.AP(g[:].tensor, 2 * (n % (N // 2)) * P,
                                          [[GHW, BANDP], [P, 2], [1, P]])
                            ro = regs[(gidx % 2) * 8 + i]
                            win = bass.AP(Th, ro, [[PROW, BANDP], [W, 2], [1, P]])
                            eng.tensor_tensor(out=win, in0=gin, in1=win, op=AT.add)

                    if "dma" not in ablate:
                        Tv = T[:].rearrange("p (rb w) -> p rb w", w=W)
                        for c in range(C):
                            nc.scalar.dma_start(
                                out=out_d[b, c].rearrange("(rb pr) w -> pr rb w", pr=M),
                                in_=Tv[c * M:(c + 1) * M, :, :],
                            )
    nc.finalize()
    return nc


def host_prep(emb, labels, tl):
    """Assemble the pre-placed patch table G and per-patch window offsets."""
    emb3 = emb.reshape(NUM_CLASSES, C, P, P)

    r0 = tl[:, :, 0].astype(np.int64)
    c0 = tl[:, :, 1].astype(np.int64)
    p0 = r0 % M
    q0 = r0 // M
    # q0 == RB-1 only at r0 == H-P (=496), where p0 == 0: no wrapped rows.
    # Shift that window down one block and use the wrap slot for the data.
    cap = (q0 == RB - 1).astype(np.int64)
    q0e = q0 - cap

    gtab = np.zeros((B, BANDP, GW), dtype=np.float32)
    jj = np.arange(P)
    pr = (p0[:, :, None] + jj) % M
    h = ((p0[:, :, None] + jj) >= M).astype(np.int64) + cap[:, :, None]
    slot = 2 * np.arange(N)[None, :, None] + h
    bb_i = np.arange(B)[:, None, None, None, None]
    part = (np.arange(C)[None, None, :, None] * M + pr[:, :, None, :])[..., None]
    col = (slot[:, :, None, :, None] * P + jj[None, None, None, None, :])
    flat = (bb_i * BANDP + part) * GW + col
    gtab.flat[flat.ravel()] = emb3[labels].ravel()

    off = np.zeros((B, 1, N), dtype=np.int32)
    off[:, 0, :] = (q0e * W + c0).astype(np.int32)
    return gtab, off


_NC_CACHE = {}


def bench_build(inputs, repeat=1, ablate=(), split=True):
    image = np.ascontiguousarray(np.asarray(inputs["image"]), dtype=np.float32)
    emb_f = np.ascontiguousarray(np.asarray(inputs["emb"]), dtype=np.float32)
    labels = np.ascontiguousarray(np.asarray(inputs["labels"])).astype(np.int64)
    tl = np.ascontiguousarray(np.asarray(inputs["top_left"])).astype(np.int64)
    gtab, off = host_prep(emb_f, labels, tl)
    nc = build_nc(repeat=repeat, ablate=ablate, split=split)
    in_maps = []
    for k in range(NCORES):
        sl = slice(k * BPC, (k + 1) * BPC)
        in_maps.append({"image": image[sl], "gtab": gtab[sl], "off": off[sl]})
    return nc, in_maps


def kernel(image, emb, labels, top_left):
    image = np.ascontiguousarray(np.asarray(image), dtype=np.float32)
    emb_f = np.ascontiguousarray(np.asarray(emb), dtype=np.float32)
    labels = np.ascontiguousarray(np.asarray(labels)).astype(np.int64)
    tl = np.ascontiguousarray(np.asarray(top_left)).astype(np.int64)

    gtab, off = host_prep(emb_f, labels, tl)

    if "nc" not in _NC_CACHE:
        _NC_CACHE["nc"] = build_nc()
    nc = _NC_CACHE["nc"]

    in_maps = []
    for k in range(NCORES):
        sl = slice(k * BPC, (k + 1) * BPC)
        in_maps.append({"image": image[sl], "gtab": gtab[sl], "off": off[sl]})
    res = run_bass_kernel_spmd(nc, in_maps, core_ids=list(range(NCORES)))
    _NC_CACHE["last_res"] = res
    out = np.concatenate([r["out"] for r in res.results], axis=0)
    return out


# revision 18
# speedup vs baseline: 3.3447x; 3.3447x over previous
"""Trainium2 Bass kernel for nn_ImagePatch: scatter-add 64 gathered 16x16x3
patches into each of 32 images [3,512,512], data-parallel over batch on 8
NeuronCores (4 images per core).

Row-interleaved layout: partition p = c*32 + (r mod 32); free dim per
partition = (q = r div 32, col) = 16 blocks x 512 cols. M=32 spans 96
partitions: SDMA engine k serves a fixed partition set (even engines cover
partitions 0-63, odd 64-127), so 96 partitions engage 12 of 16 DMA engines
vs only 8 for a 48-partition M=16 layout — measured ~20% faster end-to-end
on HW despite the G pair table costing 2x the bytes of M=16. A patch at
(r0, c0) touches 16 of the 32 residues per channel, one 16-px run each, in
row-block q0 = r0>>5 (or q0+1 for rows that wrap mod 32). Columns never
cross a boundary (c0+16 <= 512); rows need no scratch block: q0 == 15
implies r0 >= 480 hence p0 <= 16 (no wrapped rows), so the host shifts that
window down one block (reads blocks 14,15) and stores the data in the wrap
slot — the main slot adds zeros to block 14.

- The host assembles G [B, 96, 2N*16]: for patch n, row j -> partition
  (c, (p0+j)%32), slot (2n + wrapped) holds emb[label, c, j, :]; zeros
  in the other slot of the pair (pre-placed AND pre-masked).
- Per patch: ONE tensor_tensor add of a [96, 2, 16] window of T
  (row-blocks q0, q0+1 at col c0) from the static G slot pair at free
  offset 2n*16. The window offset (q0*512 + c0) comes from a register;
  offsets are group-loaded 8 per TensorLoad into two register banks so the
  sequencer issues ~2 instructions per patch instead of 3+.
- Adds split across engines by image (even: Pool/gpsimd, odd: DVE) so two
  images' add chains run concurrently; G loads issue from the SP/sync
  queue so neither adds engine's sequencer gates a later image's G
  arrival (on HW the adds, at ~700ns per register-offset op, are the
  critical path — G starvation of the tail image cost ~25%). T is
  triple-buffered (64KB columns per buf); G streams as half-tiles.

The kernel is DMA (image load/store, G table, offsets) + 64 adds per image.
The program is input-independent (compiled once, any labels/top_left values
work; adds are exact fp32).
"""

import numpy as np

import concourse.bacc as bacc
import concourse.bass as bass
import concourse.mybir as mybir
import concourse.tile as tile
from concourse.bass_utils import run_bass_kernel_spmd

B, N, C, H, W = 32, 64, 3, 512, 512
P = 16
NUM_CLASSES = 128
NCORES = 8
BPC = B // NCORES

M = 32               # row residue -> partition
RB = H // M          # 32 row-blocks in free dim
BANDP = C * M        # 48 partitions
PROW = RB * W        # 16384 elems per partition (no scratch block)
GW = 2 * N * P       # 2048 G elems per partition
GHW = GW // 2        # half-G tile width (patches 0-31 / 32-63)
RLEAD = 6            # register prefetch distance


def build_nc(repeat=1, ablate=(), split=True):
    nc = bacc.Bacc("TRN2", target_bir_lowering=False, enable_asserts=False)
    img_d = nc.dram_tensor("image", [BPC, C, H, W], mybir.dt.float32, kind="ExternalInput")
    g_d = nc.dram_tensor("gtab", [BPC, BANDP, GW], mybir.dt.float32, kind="ExternalInput")
    off_d = nc.dram_tensor("off", [BPC, 1, N], mybir.dt.int32, kind="ExternalInput")
    out_d = nc.dram_tensor("out", [BPC, C, H, W], mybir.dt.float32, kind="ExternalOutput")

    AT = mybir.AluOpType
    with tile.TileContext(nc) as tc:
        with (
            tc.tile_pool(name="big", bufs=5) as bigp,
            tc.tile_pool(name="gp", bufs=3) as gpp,
            tc.tile_pool(name="small", bufs=1) as smallp,
        ):
            vregs = [nc.vector.alloc_register(f"vreg{i}") for i in range(16)]
            pregs = [nc.gpsimd.alloc_register(f"preg{i}") for i in range(16)] if split else None

            import contextlib
            rep_ctx = tc.For_i(0, repeat, 1) if repeat > 1 else contextlib.nullcontext()
            with rep_ctx:
                offall = smallp.tile([1, BPC * N], mybir.dt.int32, tag="off")
                for b in range(BPC):
                    T = bigp.tile([BANDP, PROW], mybir.dt.float32, tag="T")
                    if "dma" not in ablate:
                        Tv = T[:].rearrange("p (rb w) -> p rb w", w=W)
                        for c in range(C):
                            nc.sync.dma_start(
                                out=Tv[c * M:(c + 1) * M, :, :],
                                in_=img_d[b, c].rearrange("(rb pr) w -> pr rb w", pr=M),
                            )
                    if b == 0:
                        # issued after the first image-load DMAs so the big
                        # transfer heads the HWDGE FIFO at kernel start
                        nc.sync.dma_start(out=offall[:],
                                          in_=off_d[:].rearrange("b one n -> one (b n)"))
                    Gt = gpp.tile([BANDP, GW], mybir.dt.float32, tag="G")
                    nc.sync.dma_start(out=Gt[:], in_=g_d[b])
                    Gh = [Gt, Gt]
                    offt = offall[:, b * N:(b + 1) * N]

                    if "adds" not in ablate:
                        if split and b % 2 == 0:
                            eng, regs = nc.gpsimd, pregs
                        else:
                            eng, regs = nc.vector, vregs
                        Th = T[:].tensor
                        # group-load 8 offsets per TensorLoad, double-buffered
                        # across two register banks of 8
                        eng.reg_load(regs[0:8], offt[0:1, 0:8])
                        for n in range(N):
                            gidx, i = n // 8, n % 8
                            if i == 0 and n + 8 < N:
                                bank = ((gidx + 1) % 2) * 8
                                eng.reg_load(regs[bank:bank + 8],
                                             offt[0:1, n + 8:n + 16])
                            gin = bass.AP(Gt[:].tensor, 2 * n * P,
                                          [[GW, BANDP], [P, 2], [1, P]])
                            ro = regs[(gidx % 2) * 8 + i]
                            win = bass.AP(Th, ro, [[PROW, BANDP], [W, 2], [1, P]])
                            eng.tensor_tensor(out=win, in0=gin, in1=win, op=AT.add)

                    if "dma" not in ablate:
                        Tv = T[:].rearrange("p (rb w) -> p rb w", w=W)
                        for c in range(C):
                            nc.scalar.dma_start(
                                out=out_d[b, c].rearrange("(rb pr) w -> pr rb w", pr=M),
                                in_=Tv[c * M:(c + 1) * M, :, :],
                            )
    nc.finalize()
    return nc


def host_prep(emb, labels, tl):
    """Assemble the pre-placed patch table G and per-patch window offsets."""
    emb3 = emb.reshape(NUM_CLASSES, C, P, P)

    r0 = tl[:, :, 0].astype(np.int64)
    c0 = tl[:, :, 1].astype(np.int64)
    p0 = r0 % M
    q0 = r0 // M
    # q0 == RB-1 only at r0 == H-P (=496), where p0 == 0: no wrapped rows.
    # Shift that window down one block and use the wrap slot for the data.
    cap = (q0 == RB - 1).astype(np.int64)
    q0e = q0 - cap

    gtab = np.zeros((B, BANDP, GW), dtype=np.float32)
    jj = np.arange(P)
    pr = (p0[:, :, None] + jj) % M
    h = ((p0[:, :, None] + jj) >= M).astype(np.int64) + cap[:, :, None]
    slot = 2 * np.arange(N)[None, :, None] + h
    bb_i = np.arange(B)[:, None, None, None, None]
    part = (np.arange(C)[None, None, :, None] * M + pr[:, :, None, :])[..., None]
    col = (slot[:, :, None, :, None] * P + jj[None, None, None, None, :])
    flat = (bb_i * BANDP + part) * GW + col
    gtab.flat[flat.ravel()] = emb3[labels].ravel()

    off = np.zeros((B, 1, N), dtype=np.int32)
    off[:, 0, :] = (q0e * W + c0).astype(np.int32)
    return gtab, off


_NC_CACHE = {}


def bench_build(inputs, repeat=1, ablate=(), split=True):
    image = np.ascontiguousarray(np.asarray(inputs["image"]), dtype=np.float32)
    emb_f = np.ascontiguousarray(np.asarray(inputs["emb"]), dtype=np.float32)
    labels = np.ascontiguousarray(np.asarray(inputs["labels"])).astype(np.int64)
    tl = np.ascontiguousarray(np.asarray(inputs["top_left"])).astype(np.int64)
    gtab, off = host_prep(emb_f, labels, tl)
    nc = build_nc(repeat=repeat, ablate=ablate, split=split)
    in_maps = []
    for k in range(NCORES):
        sl = slice(k * BPC, (k + 1) * BPC)
        in_maps.append({"image": image[sl], "gtab": gtab[sl], "off": off[sl]})
    return nc, in_maps


def kernel(image, emb, labels, top_left):
    image = np.ascontiguousarray(np.asarray(image), dtype=np.float32)
    emb_f = np.ascontiguousarray(np.asarray(emb), dtype=np.float32)
    labels = np.ascontiguousarray(np.asarray(labels)).astype(np.int64)
    tl = np.ascontiguousarray(np.asarray(top_left)).astype(np.int64)

    gtab, off = host_prep(emb_f, labels, tl)

    if "nc" not in _NC_CACHE:
        _NC_CACHE["nc"] = build_nc()
    nc = _NC_CACHE["nc"]

    in_maps = []
    for k in range(NCORES):
        sl = slice(k * BPC, (k + 1) * BPC)
        in_maps.append({"image": image[sl], "gtab": gtab[sl], "off": off[sl]})
    res = run_bass_kernel_spmd(nc, in_maps, core_ids=list(range(NCORES)))
    _NC_CACHE["last_res"] = res
    out = np.concatenate([r["out"] for r in res.results], axis=0)
    return out
